# revision 15
# baseline (speedup 1.0000x reference)
"""Trainium2 Bass kernel for nn_BagModel (segment_reduce) — fp8 DoubleRow.

Model: h = relu(x @ W1 + b1); bag_feat = segment_mean(h, ids); out = bag_feat @ W2 + b2
  x [262144, 1024] f32, ids [262144] int64 (sorted, 512 bags), W1 [1024, 512],
  b1 [512], W2 [512, 2], b2 [2]  ->  out [512, 2] f32

Strategy (8 NeuronCores, data-parallel over equal row ranges):
  - Host: split rows EQUALLY across cores (262144/8 = 32 macrotiles of 1024
    rows exactly). Bags straddling a core boundary produce partial logits on
    both cores; logits are linear in the bag sums, so the host overlap-adds
    per-core outputs (1/count, the h-scale 1/64 and W2 folded into w2b; b2
    masked to the owning core).
  - fp8 (e4m3) everywhere on the PE with MatmulPerfMode.DoubleRow: the PE
    packs 2 fp8 weights per cell and contracts two 128-row k-tiles per
    instruction at ~2x bf16 MAC throughput. W1 is pre-scaled by 64 host-side
    so its [-1/32, 1/32] range quantizes in e4m3's normal range; the kernel
    computes h64 = relu(x@(64 W1) + 64 b1) = 64*h and the 1/64 rides in w2b.
  - DRAM layouts are PE/DMA-native: xP[p, m, k, r] = x[m*1024+r, k*128+p] so
    each macrotile's x is ONE 1 MiB dma with 8 KiB contiguous per partition
    (max-bandwidth regime); per 128-row subtile the x chunk-pair IS the
    DoubleRow stationary operand. x rides the SP HWDGE ring exclusively,
    sel/b1/w2b ride the SWDGE (gpsimd) queue, and the ACT ring carries only
    the initial w1 load, so no DMA buffer-free wait can ever head-of-line
    block the relu stream or an x transfer.
  - Device, per 128-row subtile: 4 DoubleRow matmuls (k-pairs) into PSUM;
    DVE adds row-broadcast 64*b1; ScalarE relu-copies to fp8 into one half
    of a subtile-pair tile. Segment sums = DoubleRow one-hot matmuls
    (2 subtiles per instruction) accumulating into a PSUM tile that lives
    across the whole kernel, interleaved one-per-subtile with a one-macrotile
    lag so the PE never waits on a fresh relu or sel.
  - Epilogue per core: logits[b, c] = b2[c] + sum_j sums[b, j]*w2b[b, c, j]
    via DVE multiply + ScalarE accum_out reduction; DMA [64, 2] f32 out;
    host overlap-adds the 8 core outputs.
  Numerics: fp8 e4m3 inputs with f32 accumulation; numpy-model rel err vs
  the f32 reference = 5.5e-3 (absmax-relative), gate is 2e-2.
"""

import numpy as np
import ml_dtypes

N_BAGS = 512
N_CORES = 8
BPC = N_BAGS // N_CORES  # bags per core
D_IN = 1024
D_H = 512
KCH = D_IN // 128  # k-chunks of the contraction dim
MACRO = 1024  # rows per macrotile (one x DMA)
NSUB = MACRO // 128  # subtiles per macrotile
SUB = 128  # rows per subtile (one PSUM tile)
W1_SCALE = 64.0  # pow2 pre-scale so W1 quantizes in e4m3 normal range

_FP8 = ml_dtypes.float8_e4m3


def _build_nc(n_macro: int, b2vals, debug: bool = False):
    import concourse.bacc as bacc
    import concourse.mybir as mybir
    from concourse.tile import TileContext

    f32 = mybir.dt.float32
    f8 = mybir.dt.float8e4
    RELU = mybir.ActivationFunctionType.Relu
    COPY = mybir.ActivationFunctionType.Copy
    DR = mybir.MatmulPerfMode.DoubleRow

    nc = bacc.Bacc(None, target_bir_lowering=False, debug=debug)
    # xP[p, m, k, r] = x_shard[m*MACRO + r, k*128 + p]; one macrotile = one
    # DMA with KCH*MACRO = 8 KiB contiguous per partition
    xP = nc.dram_tensor("xP", [128, n_macro, KCH, MACRO], f8, kind="ExternalInput")
    # sel one-hot [row-in-subtile, subtile, local-bag-slot]; slots BPC..127
    # are zero -> rows BPC..127 of the sums PSUM tile accumulate exact zeros
    sel = nc.dram_tensor("sel", [n_macro, SUB, NSUB, SUB], f8, kind="ExternalInput")
    # w1[p, k, j] = 64 * W1[k*128 + p, j]
    w1 = nc.dram_tensor("w1", [128, KCH, D_H], f8, kind="ExternalInput")
    b1 = nc.dram_tensor("b1", [SUB, D_H], f32, kind="ExternalInput")  # 64*b1 bcast
    # w2b[b, c*D_H + j] = W2[j, c] / (64 * count[b])  (mean + h-scale folded;
    # b is a LOCAL bag slot). Cols 2*D_H + c hold b2[c] masked to the single
    # owning core so the host-side overlap-add applies b2 exactly once.
    w2b = nc.dram_tensor("w2b", [SUB, 2 * D_H + 2], f32, kind="ExternalInput")
    out = nc.dram_tensor("out", [SUB, 2], f32, kind="ExternalOutput")

    with TileContext(nc) as tc:
        with (
            tc.tile_pool(name="const", bufs=1) as cpool,
            tc.tile_pool(name="xp", bufs=4) as xpool,
            tc.tile_pool(name="selp", bufs=8) as selpool,
            tc.tile_pool(name="hp", bufs=4) as hpool,
            tc.tile_pool(name="pp", bufs=4, space="PSUM") as ppool,
            tc.tile_pool(name="sp", bufs=1, space="PSUM") as spool,
            tc.tile_pool(name="wp", bufs=1, space="PSUM") as wpool,
        ):
            # w1 on the ACT HWDGE ring in two pieces: the first matmul only
            # needs chunks 0-1, so it can start before the rest lands
            w1_t = cpool.tile([128, KCH, D_H], f8, name="w1_t")
            nc.scalar.dma_start(out=w1_t[:, 0:2, :], in_=w1[:, 0:2, :])
            nc.scalar.dma_start(out=w1_t[:, 2:KCH, :], in_=w1[:, 2:KCH, :])
            # b1 broadcast across all 128 row-partitions (bias add on DVE)
            # rides the ACT ring behind w1 (idle early); the epilogue constant
            # rides the SWDGE queue late so early bandwidth goes to x and sel
            b1_t = cpool.tile([SUB, D_H], f32, name="b1_t")
            nc.scalar.dma_start(out=b1_t[:], in_=b1[:])
            w2b_t = cpool.tile([SUB, 2 * D_H + 2], f32, name="w2b_t")

            # PE warmup: the HAM clock gate keeps the PE at 1.2 GHz until it
            # has been busy ~3.4us. The first x/w1 transfers take ~6us (cold
            # SDMA), so run dummy DoubleRow matmuls on a memset tile the
            # moment the preamble ends - by the time real data lands the PE
            # is at 2.4 GHz instead of spending its first ~14us throttled.
            warm_t = cpool.tile([128, 2, D_H], f8, name="warm_t")
            nc.vector.memset(warm_t[:], 0)
            warm_ps = wpool.tile([SUB, D_H], f32, name="warm_ps")
            for _ in range(22):
                nc.tensor.matmul(
                    warm_ps[:],
                    lhsT=warm_t[:, :, :SUB],
                    rhs=warm_t[:],
                    start=True,
                    stop=True,
                    perf_mode=DR,
                )

            sums = spool.tile([SUB, D_H], f32, name="sums")

            eligible = []  # seg items >= 2 macrotiles old
            prev1 = []  # seg items from the previous macrotile
            prev2 = []  # seg items from two macrotiles ago
            first_seg = True
            seg_total = n_macro * (NSUB // 2)
            seg_done = 0

            def emit_seg(item):
                nonlocal first_seg, seg_done
                sel_ref, sp, h2_ref = item
                seg_done += 1
                nc.tensor.matmul(
                    sums[:],
                    lhsT=sel_ref[:, 2 * sp : 2 * sp + 2, :],
                    rhs=h2_ref[:],
                    start=first_seg,
                    stop=(seg_done == seg_total),
                    perf_mode=DR,
                    skip_group_check=True,
                )
                first_seg = False

            for m in range(n_macro):
                eligible.extend(prev2)
                prev2 = prev1
                prev1 = []
                x_t = xpool.tile([128, KCH, MACRO], f8, name="x_t")
                # ALL x on the SP HWDGE ring, whose sequencer does nothing
                # else: an x slot-free wait can then never head-of-line-block
                # the ACT sequencer (which must keep streaming relus). First
                # macrotiles in finer k-pieces so the first matmuls start as
                # soon as the first chunk-pair lands.
                # m=1 rides the ACT ring: early only, while the ring is
                # otherwise done (w1+b1) and no relus exist yet - parallelizes
                # the cold-SDMA window so m0/m1 land concurrently. From m>=2
                # the ACT ring never sees another DMA.
                xeng = nc.scalar if m == 1 else nc.sync
                if m == 0:
                    pieces = [(0, 1), (1, 1), (2, 2), (4, 2), (6, 2)]
                elif m == 1:
                    pieces = [(0, 2), (2, 2), (4, 2), (6, 2)]
                elif m == 2:
                    pieces = [(0, 4), (4, 4)]
                else:
                    pieces = [(0, KCH)]
                for ks, kstep in pieces:
                    xeng.dma_start(
                        out=x_t[:, ks : ks + kstep, :],
                        in_=xP[:, m, ks : ks + kstep, :],
                    )
                sel_t = selpool.tile([SUB, NSUB, SUB], f8, name="sel_t")
                nc.gpsimd.dma_start(out=sel_t[:], in_=sel[m])
                if m == min(12, n_macro - 1):
                    nc.gpsimd.dma_start(out=w2b_t[:], in_=w2b[:])

                h2 = None
                fresh = []
                for s in range(NSUB):
                    h_ps = ppool.tile([SUB, D_H], f32, name="h_ps")
                    for kp in range(KCH // 2):
                        nc.tensor.matmul(
                            h_ps[:],
                            lhsT=x_t[:, 2 * kp : 2 * kp + 2, s * SUB : (s + 1) * SUB],
                            rhs=w1_t[:, 2 * kp : 2 * kp + 2, :],
                            start=(kp == 0),
                            stop=(kp == KCH // 2 - 1),
                            perf_mode=DR,
                        )
                    hb = hpool.tile([SUB, D_H], f32, name="hb", tag="hb", bufs=4)
                    nc.vector.tensor_add(hb[:], h_ps[:], b1_t[:])
                    if s % 2 == 0:
                        h2 = hpool.tile(
                            [SUB, 2, D_H], f8, name="h2", tag="h2", bufs=14
                        )
                    nc.scalar.activation(h2[:, s % 2, :], hb[:], RELU)
                    if s % 2 == 1:
                        fresh.append((sel_t, s // 2, h2))
                    # interleave TWO-macrotile-old segment matmuls into this
                    # macrotile's main stream (one per subtile): their sel/h2
                    # deps are long satisfied, so the PE never stalls on a
                    # fresh relu or a late sel DMA, and sel buffers free
                    # smoothly instead of in bursts.
                    if eligible:
                        emit_seg(eligible.pop(0))
                prev1 = fresh

            # drain the last macrotiles' segment matmuls (the final one
            # waits on the last subtile's bias/relu chain - ~1.5us, once)
            for item in eligible + prev2 + prev1:
                emit_seg(item)

            logits = cpool.tile([SUB, 2], f32, name="logits")
            for c in range(2):
                # multiply on DVE, row-reduce via ScalarE accum_out so the
                # two class reductions pipeline across engines
                scr = cpool.tile([SUB, D_H], f32, name=f"scr{c}")
                nc.vector.tensor_mul(
                    scr[:], sums[:], w2b_t[:, c * D_H : (c + 1) * D_H]
                )
                scr2 = cpool.tile([SUB, D_H], f32, name=f"scr2{c}")
                red = cpool.tile([SUB, 1], f32, name=f"red{c}")
                nc.scalar.activation(scr2[:], scr[:], COPY, accum_out=red[:])
                nc.vector.tensor_add(
                    logits[:, c : c + 1],
                    red[:],
                    w2b_t[:, 2 * D_H + c : 2 * D_H + c + 1],
                )
            nc.sync.dma_start(out=out[:], in_=logits[:])
    nc.finalize()
    return nc


def _prepare_inputs(x, ids, W1, b1, W2, b2):
    """Equal row split across cores; local bag slots; fp8 DRAM layouts.

    Core k gets rows [k*R, (k+1)*R). A bag straddling a core boundary gets
    partial logits on both cores; since logits are linear in the bag sums
    (1/count, 1/64 and W2 folded host-side), the host overlap-adds the
    per-core outputs. b2 rides along masked to the owning core.
    Returns (in_maps, n_macro, first_bag, nloc) for the gather.
    """
    ids = np.asarray(ids).astype(np.int64)
    x = np.asarray(x, dtype=np.float32)
    n = x.shape[0]

    R = -(-n // N_CORES)  # rows per core
    n_macro = max(1, -(-R // MACRO))
    L = n_macro * MACRO

    counts = np.bincount(ids, minlength=N_BAGS).astype(np.float64)
    recip_all = np.where(counts > 0, 1.0 / (counts * W1_SCALE), 0.0).astype(
        np.float32
    )
    first_occ = np.searchsorted(ids, np.arange(N_BAGS))  # first row of each bag

    w1s = np.asarray(W1, dtype=np.float32) * W1_SCALE
    w1P = np.ascontiguousarray(
        w1s.reshape(KCH, 128, D_H).transpose(1, 0, 2)
    ).astype(_FP8)
    b1_bc = np.ascontiguousarray(
        np.broadcast_to(
            np.asarray(b1, dtype=np.float32)[None, :] * W1_SCALE, (SUB, D_H)
        )
    )
    W2f = np.asarray(W2, dtype=np.float32)
    b2f = np.asarray(b2, dtype=np.float32).reshape(2)

    in_maps = []
    first_bag = np.zeros(N_CORES, dtype=np.int64)
    nloc = np.zeros(N_CORES, dtype=np.int64)
    for k in range(N_CORES):
        lo, hi = k * R, min((k + 1) * R, n)
        nk = hi - lo
        # xP[p, m, kc, r] = shard[m*MACRO + r, kc*128 + p]
        if nk == L:
            shard8 = x[lo:hi].astype(_FP8)
        else:
            shard8 = np.zeros((L, D_IN), dtype=_FP8)
            if nk:
                shard8[:nk] = x[lo:hi].astype(_FP8)
        xP_k = np.ascontiguousarray(
            shard8.reshape(n_macro, MACRO, KCH, 128).transpose(3, 0, 2, 1)
        )

        g0 = int(ids[lo]) if nk else 0
        first_bag[k] = g0
        sel_k = np.zeros((n_macro, SUB, NSUB, SUB), dtype=_FP8)
        if nk:
            r = np.arange(nk)
            lb = ids[lo:hi] - g0  # local bag slot
            assert lb.max() < SUB, "core spans more than 128 bags"
            nloc[k] = int(lb.max()) + 1
            mi = r // MACRO
            pi = r % SUB
            si = (r % MACRO) // SUB
            sel_k[mi, pi, si, lb] = 1.0

        # local slot b -> global bag g0+b (slots beyond nloc stay zero)
        w2b_k = np.zeros((SUB, 2 * D_H + 2), dtype=np.float32)
        nl = int(nloc[k])
        gl = np.arange(g0, min(g0 + nl, N_BAGS))
        rk = recip_all[gl]  # [nl]
        for c in range(2):
            w2b_k[: len(gl), c * D_H : (c + 1) * D_H] = (
                rk[:, None] * W2f[:, c][None, :]
            )
            # b2 applied only by the core owning the bag's first row
            own = (first_occ[gl] >= lo) & (first_occ[gl] < hi)
            w2b_k[: len(gl), 2 * D_H + c] = np.where(own, b2f[c], 0.0)

        in_maps.append(
            {
                "xP": xP_k,
                "sel": sel_k,
                "w1": w1P,
                "b1": b1_bc,
                "w2b": w2b_k,
            }
        )
    return in_maps, n_macro, first_bag, nloc


def _run(x, ids, W1, b1, W2, b2, trace=False, trace_kwargs=None):
    from concourse.bass_utils import run_bass_kernel_spmd

    in_maps, n_macro, first_bag, nloc = _prepare_inputs(x, ids, W1, b1, W2, b2)
    b2f = np.asarray(b2, dtype=np.float32).reshape(2)
    nc = _build_nc(n_macro, b2f)
    res = run_bass_kernel_spmd(
        nc,
        in_maps,
        list(range(N_CORES)),
        trace=trace,
        **(trace_kwargs or {}),
    )
    full = np.zeros((N_BAGS, 2), dtype=np.float32)
    for k in range(N_CORES):
        out_k = np.asarray(res.results[k]["out"], dtype=np.float32)
        g0, nl = int(first_bag[k]), int(nloc[k])
        nl = min(nl, N_BAGS - g0)
        full[g0 : g0 + nl] += out_k[:nl]
    return full, res


def kernel(x, ids, W1, b1, W2, b2):
    out, _ = _run(x, ids, W1, b1, W2, b2, trace=False)
    return out
